# revision 38
# baseline (speedup 1.0000x reference)
"""NCC loss (9x9x9 box normalized cross-correlation) on 8 TRN2 NeuronCores.

Inputs: y_pred, y_true f32 (2,1,128,128,128). Output: scalar f32 loss.

Sharding: D axis (dim 2) split 4-ways per batch -> 8 slabs of 32 D-slices,
each with a 4-slice halo (host zero-pads volume edges). Inputs are converted
to bf16 on the host (same precision as the on-device copy the previous
version did) and packed into a [104, 64, 128] layout: partitions 0..39 hold
the 40 halo'd d-rows for h-block 0 (h 0..63), partitions 64..103 hold them
for h-block 1 (h 64..127), rows 40..63 / 104..127 are zero.

Per core, separable box filter as three matmul passes (contract D, then W,
then H) so every intermediate is a full-128-partition tile:

  prep  : I*I (DVE), J*J (DVE), I*J (Pool) products in bf16
  P_D   : per h, lhsT=vol[d,w] slab, rhs=banded BD[40,32] -> t1 [w,(h,d')]
  P_W   : per d', lhsT=t1[w,h], rhs=band BW[128,128]      -> t2 [h,(d',w')]
  P_H   : stationary band BH, rhs=t2 chunks of 512        -> PSUM [h',512]
  ptw   : cc = cross^2/(I_var*J_var); the three big PSUM subtractions are
          done ON THE PE via accumulating -identity matmuls, reciprocal via
          the fast bit-trick custom DVE op, final mean via ones-matmul
          reduction accumulated in PSUM.

Scheduling: P_D and P_W share one 8-bank PSUM ring and interleave per
volume; each chunk's early stage (box(I)/box(J) P_H matmuls + ap/bp +
qI/qJ/m) is hoisted into that stream since it only needs the I/J volumes'
t2, leaving phase 3 just the I2/J2/IJ matmuls + subtract + divide tail,
software-pipelined two chunks deep. PSUM evacuations alternate ACT/DVE
(Pool cannot address PSUM); Pool carries the SBUF-only products.
Host: sum per-core [128,8] partials, loss = -sum / N.
"""

import math

import numpy as np
import ml_dtypes

import concourse.bacc as bacc
import concourse.tile as tile
from concourse import mybir
from concourse.bass_utils import run_bass_kernel_spmd

# Custom fused DVE op: out = rec * cross^2, accum_out = per-partition sum.
# Registered into concourse.dve_ops at import so table-gen and codegen see it.
from operator import add as _add
from concourse.dve_spec import Spec as _Spec, Src0 as _S0, Src1 as _S1, \
    Zero as _Zero, sq as _sq
from concourse import dve_ops as _dve_ops


def _ref_ccsq(in0, in1, s0, s1, imm2):
    b = (in0.astype(np.float32) * in1.astype(np.float32) ** 2)
    b = b.astype(np.float32)
    return b, b.reshape(b.shape[0], -1).sum(axis=-1, keepdims=True)


if "CC_SQ_ANT" not in _dve_ops._SUB_OPCODE_FOR_NAME:
    _row = _dve_ops._CUSTOM_DVE_ROW_BASE + len(_dve_ops.OPS)
    _spec = _Spec(body=_S0 * _sq(_S1), accum=_add, accum_init=_Zero,
                  reference=_ref_ccsq)
    CC_SQ_ANT = _dve_ops.DveOp(
        "CC_SQ_ANT", _spec, subdim=False,
        uops_sha={"v3": "f2c663096c4a0563", "v4": "6187c63a2129eae0"})
    _dve_ops.OPS.append(CC_SQ_ANT)
    _dve_ops.CUSTOM_DVE_SPECS["CC_SQ_ANT"] = _spec
    _dve_ops._SUB_OPCODE_FOR_NAME["CC_SQ_ANT"] = _row
else:  # module re-import
    CC_SQ_ANT = next(o for o in _dve_ops.OPS if o.name == "CC_SQ_ANT")

F32 = mybir.dt.float32
BF16 = mybir.dt.bfloat16
ALU = mybir.AluOpType
ACTF = mybir.ActivationFunctionType

B, D, H, W = 2, 128, 128, 128
DL, PAD = 32, 4
DH = DL + 2 * PAD            # 40
SQS = math.sqrt(1.0 / 729.0)
N_TOT = float(B * D * H * W)

_CACHE = {}


def _build():
    nc = bacc.Bacc(trn_type="TRN2", target_bir_lowering=False)

    i_dram = nc.dram_tensor("i_pk", [104, 64, 128], BF16, kind="ExternalInput")
    j_dram = nc.dram_tensor("j_pk", [104, 64, 128], BF16, kind="ExternalInput")
    out_dram = nc.dram_tensor("partials", [128, 8], F32, kind="ExternalOutput")

    with tile.TileContext(nc) as tc:
        with (
            tc.tile_pool(name="bands", bufs=1) as bands,
            tc.tile_pool(name="stage", bufs=1) as stage,
            tc.tile_pool(name="accp", bufs=1) as accp,
        ):
            # ---------- band / constant matrices ----------
            # BD[p, j] = 1 iff j <= p <= j+8, duplicated at partition 64.
            bd = bands.tile([104, 32], BF16)
            nc.gpsimd.memset(bd[0:40, :], 1.0)
            nc.gpsimd.affine_select(bd[0:40, :], bd[0:40, :], pattern=[[-1, 32]],
                                    compare_op=ALU.is_ge, fill=0.0,
                                    base=0, channel_multiplier=1)
            nc.gpsimd.affine_select(bd[0:40, :], bd[0:40, :], pattern=[[1, 32]],
                                    compare_op=ALU.is_ge, fill=0.0,
                                    base=8, channel_multiplier=-1)

            # BW = BH: [p, j] = 1 iff |p - j| <= 4
            bw = bands.tile([128, 128], BF16)
            nc.gpsimd.memset(bw[:, :], 1.0)
            nc.gpsimd.affine_select(bw[:, :], bw[:, :], pattern=[[-1, 128]],
                                    compare_op=ALU.is_ge, fill=0.0,
                                    base=PAD, channel_multiplier=1)
            nc.gpsimd.affine_select(bw[:, :], bw[:, :], pattern=[[1, 128]],
                                    compare_op=ALU.is_ge, fill=0.0,
                                    base=PAD, channel_multiplier=-1)

            # -identity for PE-side subtraction
            negI = bands.tile([128, 128], BF16)
            nc.gpsimd.memset(negI[:, :], -1.0)
            nc.gpsimd.affine_select(negI[:, :], negI[:, :], pattern=[[-1, 128]],
                                    compare_op=ALU.is_ge, fill=0.0,
                                    base=0, channel_multiplier=1)
            nc.gpsimd.affine_select(negI[:, :], negI[:, :], pattern=[[1, 128]],
                                    compare_op=ALU.is_ge, fill=0.0,
                                    base=0, channel_multiplier=-1)

            ones = bands.tile([128, 1], BF16)
            nc.gpsimd.memset(ones[:, :], 1.0)

            # t2 tiles live until the end
            t2 = [stage.tile([128, 32, 128], BF16, name=f"t2_{v}")
                  for v in range(5)]

            # ---------- inputs + products ----------
            # pool stack (LIFO): t1 -> psD -> vols; vols popped after P_D.
            t1p = tc.tile_pool(name="t1", bufs=1)
            t1pool = t1p.__enter__()
            t1 = [t1pool.tile([128, 128, 32], BF16, name=f"t1_{v}")
                  for v in range(5)]
            esp = tc.tile_pool(name="esp", bufs=8)
            espp = esp.__enter__()
            ptw2p = tc.tile_pool(name="ptw2", bufs=4)
            ptw2 = ptw2p.__enter__()
            psD = tc.tile_pool(name="psD", bufs=4, space="PSUM")
            psDp = psD.__enter__()
            # shared 4KB-per-partition PSUM ring used by both P_D and P_W
            volp = tc.tile_pool(name="vols", bufs=1)
            vols = volp.__enter__()
            vi = vols.tile([104, 64, 128], BF16, name="vi")
            vj = vols.tile([104, 64, 128], BF16, name="vj")
            vi2 = vols.tile([104, 64, 128], BF16, name="vi2")
            vj2 = vols.tile([104, 64, 128], BF16, name="vj2")
            vij = vols.tile([104, 64, 128], BF16, name="vij")
            for q in range(8):
                s = slice(q * 8, q * 8 + 8)
                nc.sync.dma_start(out=vi[:, s, :], in_=i_dram[:, s, :])
                if q == 1:
                    nc.sync.dma_start(out=bd[64:104, :], in_=bd[0:40, :])
            for q in range(8):
                s = slice(q * 8, q * 8 + 8)
                nc.sync.dma_start(out=vj[:, s, :], in_=j_dram[:, s, :])

            nev = 0

            def evac_on_act(n):
                return n % 2 == 0

            def p_d(v, vol):
                nonlocal nev
                for hb in range(4):           # 2-bank tiles of 32 h
                    ps = psDp.tile([128, 1024], F32, tag="psD",
                                   name="psd").rearrange(
                        "p (a b) -> p a b", a=32, b=32)
                    for k in range(32):
                        h = hb * 32 + k
                        b, hl = h >> 6, h & 63
                        nc.tensor.matmul(
                            out=ps[:, k, :],
                            lhsT=vol[64 * b:64 * b + 40, hl, :],
                            rhs=bd[64 * b:64 * b + 40, :])
                    dst = t1[v][:, hb * 32:hb * 32 + 32, :]
                    if evac_on_act(nev):
                        nc.scalar.copy(dst, ps[:, :, :])
                    else:
                        nc.vector.tensor_copy(dst, ps[:, :, :])
                    nev += 1

            def p_w(v):
                nonlocal nev
                for db in range(4):
                    ps = psDp.tile([128, 1024], F32, tag="psD",
                                   name="psw").rearrange(
                        "p (a b) -> p a b", a=8, b=128)
                    for k in range(8):
                        dp = db * 8 + k
                        nc.tensor.matmul(out=ps[:, k, :],
                                         lhsT=t1[v][:, :, dp],
                                         rhs=bw[:, :])
                    dst = t2[v][:, db * 8:db * 8 + 8, :]
                    if evac_on_act(nev):
                        nc.scalar.copy(dst, ps[:, :, :])
                    else:
                        nc.vector.tensor_copy(dst, ps[:, :, :])
                    nev += 1

            # Early per-chunk stage: psI/psJ + ap/bp + qI/qJ/m only needs
            # the I and J volumes' t2 -> hoisted into the P_D/P_W stream.
            qtiles = {}

            def early_chunk(c):
                rhsI = t2[0][:, c * 4:c * 4 + 4, :].rearrange(
                    "p a b -> p (a b)")
                rhsJ = t2[1][:, c * 4:c * 4 + 4, :].rearrange(
                    "p a b -> p (a b)")
                ps = psDp.tile([128, 1024], F32, tag="psD", name="psij")
                nc.tensor.matmul(out=ps[:, 0:512], lhsT=bw[:, :], rhs=rhsI)
                nc.tensor.matmul(out=ps[:, 512:1024], lhsT=bw[:, :], rhs=rhsJ)
                ap = ptw2.tile([128, 512], BF16, tag="ap", name="ap")
                bp = ptw2.tile([128, 512], BF16, tag="bp", name="bp")
                nc.scalar.mul(ap[:, :], ps[:, 0:512], SQS)
                nc.scalar.mul(bp[:, :], ps[:, 512:1024], SQS)
                qI = espp.tile([128, 512], BF16, tag="qI", name="qI")
                qJ = espp.tile([128, 512], BF16, tag="qJ", name="qJ")
                m = espp.tile([128, 512], BF16, tag="m", name="m")
                nc.gpsimd.tensor_tensor(out=qI[:, :], in0=ap[:, :],
                                        in1=ap[:, :], op=ALU.mult)
                nc.scalar.square(qJ[:, :], bp[:, :])
                nc.gpsimd.tensor_tensor(out=m[:, :], in0=ap[:, :],
                                        in1=bp[:, :], op=ALU.mult)
                qtiles[c] = (qI, qJ, m)

            # Raw inputs first (only DMA-gated), products trail their prep.
            p_d(0, vi)
            p_d(1, vj)
            p_w(0)
            for q in range(4):
                s = slice(q * 16, q * 16 + 16)
                nc.scalar.square(vi2[:, s, :], vi[:, s, :])
                nc.vector.tensor_tensor(out=vj2[:, s, :], in0=vj[:, s, :],
                                        in1=vj[:, s, :], op=ALU.mult)
                nc.gpsimd.tensor_tensor(out=vij[:, s, :], in0=vi[:, s, :],
                                        in1=vj[:, s, :], op=ALU.mult)
            p_d(2, vi2)
            p_w(1)
            early_chunk(0)
            early_chunk(1)
            p_d(3, vj2)
            p_w(2)
            early_chunk(2)
            early_chunk(3)
            p_d(4, vij)
            p_w(3)
            early_chunk(4)
            early_chunk(5)
            p_w(4)
            early_chunk(6)
            early_chunk(7)

            volp.__exit__(None, None, None)

            # ---------- P_W + P_H + pointwise, pipelined per d'-block ----
            # P_W ordered d'-block-outer so chunk c's P_H + pointwise can
            # trail one block behind P_W(c+1): Pool's pointwise overlaps
            # ACT/DVE evacuations, P_W matmuls fill PE between P_H chunks.
            ptwp = tc.tile_pool(name="ptw", bufs=4)
            ptw = ptwp.__enter__()

            accs = accp.tile([128, 8], F32)
            pend = {}

            def emit_chunk(c):
                rhs = [t2[v][:, c * 4:c * 4 + 4, :].rearrange(
                    "p a b -> p (a b)") for v in (2, 3, 4)]
                qI, qJ, m = qtiles[c]
                tA = psDp.tile([128, 1024], F32, tag="psD", name="psA")
                tB = psDp.tile([128, 1024], F32, tag="psD", name="psB")
                psI2 = tA[:, 0:512]
                psJ2 = tA[:, 512:1024]
                psIJ = tB[:, 0:512]
                nc.tensor.matmul(out=psI2[:, :], lhsT=bw[:, :], rhs=rhs[0],
                                 start=True, stop=False)
                nc.tensor.matmul(out=psI2[:, :], lhsT=negI[:, :], rhs=qI[:, :],
                                 start=False, stop=True)
                nc.tensor.matmul(out=psJ2[:, :], lhsT=bw[:, :], rhs=rhs[1],
                                 start=True, stop=False)
                nc.tensor.matmul(out=psJ2[:, :], lhsT=negI[:, :], rhs=qJ[:, :],
                                 start=False, stop=True)
                nc.tensor.matmul(out=psIJ[:, :], lhsT=bw[:, :], rhs=rhs[2],
                                 start=True, stop=False)
                nc.tensor.matmul(out=psIJ[:, :], lhsT=negI[:, :], rhs=m[:, :],
                                 start=False, stop=True)
                pend[c] = (psI2, psJ2, psIJ)

            def finish_chunk(c):
                psI2, psJ2, psIJ = pend.pop(c)
                qtiles.pop(c)
                ivp = ptw.tile([128, 512], BF16, tag="ivp", name="ivp")
                crp = ptw.tile([128, 512], BF16, tag="crp", name="crp")
                nc.scalar.copy(ivp[:, :], psI2[:, :])
                nc.scalar.copy(crp[:, :], psIJ[:, :])

                den = ptw.tile([128, 512], F32, tag="den", name="den")
                nc.vector.tensor_tensor(out=den[:, :], in0=psJ2[:, :],
                                        in1=ivp[:, :], op=ALU.mult)
                rec = ptw.tile([128, 512], F32, tag="rec", name="rec")
                nc.vector.reciprocal_approx_fast(out=rec[:, :], in_=den[:, :])

                cc = ptw.tile([128, 512], F32, tag="cc", name="cc")
                nc.vector._custom_dve(CC_SQ_ANT, out=cc[:, :], in0=rec[:, :],
                                      in1=crp[:, :],
                                      accum_out=accs[:, c:c + 1])

            for c in range(9):
                if c < 8:
                    emit_chunk(c)
                if c >= 1:
                    finish_chunk(c - 1)

            nc.sync.dma_start(out=out_dram[:, :], in_=accs[:, :])
            ptwp.__exit__(None, None, None)
            psD.__exit__(None, None, None)
            ptw2p.__exit__(None, None, None)
            esp.__exit__(None, None, None)
            t1p.__exit__(None, None, None)

    nc.compile()
    return nc


def kernel(y_pred: np.ndarray, y_true: np.ndarray) -> np.ndarray:
    y_pred = np.asarray(y_pred, dtype=np.float32)
    y_true = np.asarray(y_true, dtype=np.float32)

    if "nc" not in _CACHE:
        _CACHE["nc"] = _build()
    nc = _CACHE["nc"]

    ib = y_true.astype(ml_dtypes.bfloat16)
    jb = y_pred.astype(ml_dtypes.bfloat16)

    in_maps = []
    for core in range(8):
        b = core // 4
        d0 = (core % 4) * DL
        lo, hi = d0 - PAD, d0 + DL + PAD
        slo, shi = max(lo, 0), min(hi, D)
        ipk = np.zeros((104, 64, 128), ml_dtypes.bfloat16)
        jpk = np.zeros((104, 64, 128), ml_dtypes.bfloat16)
        for hb in range(2):
            hs = slice(hb * 64, hb * 64 + 64)
            p0 = 64 * hb
            ipk[p0 + slo - lo:p0 + shi - lo] = ib[b, 0, slo:shi, hs, :]
            jpk[p0 + slo - lo:p0 + shi - lo] = jb[b, 0, slo:shi, hs, :]
        in_maps.append({"i_pk": ipk, "j_pk": jpk})

    res = run_bass_kernel_spmd(nc, in_maps, core_ids=list(range(8)))
    total = 0.0
    for r in res.results:
        total += float(np.asarray(r["partials"], np.float64).sum())
    return np.float32(-total / N_TOT)


if __name__ == "__main__":
    rng = np.random.default_rng(0)
    yp = rng.standard_normal((B, 1, D, H, W), dtype=np.float32)
    yt = rng.standard_normal((B, 1, D, H, W), dtype=np.float32)
    print("loss:", kernel(yp, yt))


# revision 40
# speedup vs baseline: 1.0782x; 1.0782x over previous
"""NCC loss (9x9x9 box normalized cross-correlation) on 8 TRN2 NeuronCores.

Inputs: y_pred, y_true f32 (2,1,128,128,128). Output: scalar f32 loss.

Sharding: D axis (dim 2) split 4-ways per batch -> 8 slabs of 32 D-slices,
each with a 4-slice halo (host zero-pads volume edges). Inputs are converted
to bf16 on the host (same precision as the on-device copy the previous
version did) and packed into a [104, 64, 128] layout: partitions 0..39 hold
the 40 halo'd d-rows for h-block 0 (h 0..63), partitions 64..103 hold them
for h-block 1 (h 64..127), rows 40..63 / 104..127 are zero.

Per core, separable box filter as three matmul passes (contract D, then W,
then H) so every intermediate is a full-128-partition tile:

  prep  : I*I (DVE), J*J (DVE), I*J (Pool) products in bf16
  P_D   : per h, lhsT=vol[d,w] slab, rhs=banded BD[40,32] -> t1 [w,(h,d')]
  P_W   : per d', lhsT=t1[w,h], rhs=band BW[128,128]      -> t2 [h,(d',w')]
  P_H   : stationary band BH, rhs=t2 chunks of 512        -> PSUM [h',512]
  ptw   : cc = cross^2/(I_var*J_var); the three big PSUM subtractions are
          done ON THE PE via accumulating -identity matmuls, reciprocal via
          the fast bit-trick custom DVE op, final mean via ones-matmul
          reduction accumulated in PSUM.

Scheduling: P_D and P_W share one 8-bank PSUM ring and interleave per
volume; each chunk's early stage (box(I)/box(J) P_H matmuls + ap/bp +
qI/qJ/m) is hoisted into that stream since it only needs the I/J volumes'
t2, leaving phase 3 just the I2/J2/IJ matmuls + subtract + divide tail,
software-pipelined two chunks deep. PSUM evacuations alternate ACT/DVE
(Pool cannot address PSUM); Pool carries the SBUF-only products.
Host: sum per-core [128,8] partials, loss = -sum / N.
"""

import math

import numpy as np
import ml_dtypes

import concourse.bacc as bacc
import concourse.tile as tile
from concourse import mybir
from concourse.bass_utils import run_bass_kernel_spmd

# Custom fused DVE op: out = rec * cross^2, accum_out = per-partition sum.
# Registered into concourse.dve_ops at import so table-gen and codegen see it.
from operator import add as _add
from concourse.dve_spec import Spec as _Spec, Src0 as _S0, Src1 as _S1, \
    Zero as _Zero, sq as _sq
from concourse import dve_ops as _dve_ops


def _ref_ccsq(in0, in1, s0, s1, imm2):
    b = (in0.astype(np.float32) * in1.astype(np.float32) ** 2)
    b = b.astype(np.float32)
    return b, b.reshape(b.shape[0], -1).sum(axis=-1, keepdims=True)


if "CC_SQ_ANT" not in _dve_ops._SUB_OPCODE_FOR_NAME:
    _row = _dve_ops._CUSTOM_DVE_ROW_BASE + len(_dve_ops.OPS)
    _spec = _Spec(body=_S0 * _sq(_S1), accum=_add, accum_init=_Zero,
                  reference=_ref_ccsq)
    CC_SQ_ANT = _dve_ops.DveOp(
        "CC_SQ_ANT", _spec, subdim=False,
        uops_sha={"v3": "f2c663096c4a0563", "v4": "6187c63a2129eae0"})
    _dve_ops.OPS.append(CC_SQ_ANT)
    _dve_ops.CUSTOM_DVE_SPECS["CC_SQ_ANT"] = _spec
    _dve_ops._SUB_OPCODE_FOR_NAME["CC_SQ_ANT"] = _row
else:  # module re-import
    CC_SQ_ANT = next(o for o in _dve_ops.OPS if o.name == "CC_SQ_ANT")

F32 = mybir.dt.float32
BF16 = mybir.dt.bfloat16
ALU = mybir.AluOpType
ACTF = mybir.ActivationFunctionType

B, D, H, W = 2, 128, 128, 128
DL, PAD = 32, 4
DH = DL + 2 * PAD            # 40
SQS = math.sqrt(1.0 / 729.0)
N_TOT = float(B * D * H * W)

_CACHE = {}


def _build():
    nc = bacc.Bacc(trn_type="TRN2", target_bir_lowering=False)

    i_dram = nc.dram_tensor("i_pk", [104, 64, 128], BF16, kind="ExternalInput")
    j_dram = nc.dram_tensor("j_pk", [104, 64, 128], BF16, kind="ExternalInput")
    out_dram = nc.dram_tensor("partials", [128, 8], F32, kind="ExternalOutput")

    with tile.TileContext(nc) as tc:
        with (
            tc.tile_pool(name="bands", bufs=1) as bands,
            tc.tile_pool(name="stage", bufs=1) as stage,
            tc.tile_pool(name="accp", bufs=1) as accp,
        ):
            # ---------- band / constant matrices ----------
            # BD[p, j] = 1 iff j <= p <= j+8, duplicated at partition 64.
            bd = bands.tile([104, 32], BF16)
            nc.gpsimd.memset(bd[0:40, :], 1.0)
            nc.gpsimd.affine_select(bd[0:40, :], bd[0:40, :], pattern=[[-1, 32]],
                                    compare_op=ALU.is_ge, fill=0.0,
                                    base=0, channel_multiplier=1)
            nc.gpsimd.affine_select(bd[0:40, :], bd[0:40, :], pattern=[[1, 32]],
                                    compare_op=ALU.is_ge, fill=0.0,
                                    base=8, channel_multiplier=-1)

            # BW = BH: [p, j] = 1 iff |p - j| <= 4
            bw = bands.tile([128, 128], BF16)
            nc.gpsimd.memset(bw[:, :], 1.0)
            nc.gpsimd.affine_select(bw[:, :], bw[:, :], pattern=[[-1, 128]],
                                    compare_op=ALU.is_ge, fill=0.0,
                                    base=PAD, channel_multiplier=1)
            nc.gpsimd.affine_select(bw[:, :], bw[:, :], pattern=[[1, 128]],
                                    compare_op=ALU.is_ge, fill=0.0,
                                    base=PAD, channel_multiplier=-1)

            # -identity for PE-side subtraction
            negI = bands.tile([128, 128], BF16)
            nc.gpsimd.memset(negI[:, :], -1.0)
            nc.gpsimd.affine_select(negI[:, :], negI[:, :], pattern=[[-1, 128]],
                                    compare_op=ALU.is_ge, fill=0.0,
                                    base=0, channel_multiplier=1)
            nc.gpsimd.affine_select(negI[:, :], negI[:, :], pattern=[[1, 128]],
                                    compare_op=ALU.is_ge, fill=0.0,
                                    base=0, channel_multiplier=-1)

            ones = bands.tile([128, 1], BF16)
            nc.gpsimd.memset(ones[:, :], 1.0)

            # t2 tiles live until the end
            t2 = [stage.tile([128, 32, 128], BF16, name=f"t2_{v}")
                  for v in range(5)]

            # ---------- inputs + products ----------
            # pool stack (LIFO): t1 -> psD -> vols; vols popped after P_D.
            t1p = tc.tile_pool(name="t1", bufs=1)
            t1pool = t1p.__enter__()
            t1 = [t1pool.tile([128, 128, 32], BF16, name=f"t1_{v}")
                  for v in range(5)]
            esp = tc.tile_pool(name="esp", bufs=8)
            espp = esp.__enter__()
            ptw2p = tc.tile_pool(name="ptw2", bufs=4)
            ptw2 = ptw2p.__enter__()
            psD = tc.tile_pool(name="psD", bufs=4, space="PSUM")
            psDp = psD.__enter__()
            # shared 4KB-per-partition PSUM ring used by both P_D and P_W
            volp = tc.tile_pool(name="vols", bufs=1)
            vols = volp.__enter__()
            vi = vols.tile([104, 64, 128], BF16, name="vi")
            vj = vols.tile([104, 64, 128], BF16, name="vj")
            vi2 = vols.tile([104, 64, 128], BF16, name="vi2")
            vj2 = vols.tile([104, 64, 128], BF16, name="vj2")
            vij = vols.tile([104, 64, 128], BF16, name="vij")
            for q in range(8):
                s = slice(q * 8, q * 8 + 8)
                nc.sync.dma_start(out=vi[:, s, :], in_=i_dram[:, s, :])
                if q == 1:
                    nc.sync.dma_start(out=bd[64:104, :], in_=bd[0:40, :])
            for q in range(8):
                s = slice(q * 8, q * 8 + 8)
                nc.sync.dma_start(out=vj[:, s, :], in_=j_dram[:, s, :])

            nev = 0

            def evac_on_act(n):
                return n % 2 == 1

            def p_d(v, vol):
                nonlocal nev
                for hb in range(4):           # 2-bank tiles of 32 h
                    ps = psDp.tile([128, 1024], F32, tag="psD",
                                   name="psd").rearrange(
                        "p (a b) -> p a b", a=32, b=32)
                    for k in range(32):
                        h = hb * 32 + k
                        b, hl = h >> 6, h & 63
                        nc.tensor.matmul(
                            out=ps[:, k, :],
                            lhsT=vol[64 * b:64 * b + 40, hl, :],
                            rhs=bd[64 * b:64 * b + 40, :])
                    dst = t1[v][:, hb * 32:hb * 32 + 32, :]
                    if evac_on_act(nev):
                        nc.scalar.copy(dst, ps[:, :, :])
                    else:
                        nc.vector.tensor_copy(dst, ps[:, :, :])
                    nev += 1

            def p_w(v):
                nonlocal nev
                for db in range(4):
                    ps = psDp.tile([128, 1024], F32, tag="psD",
                                   name="psw").rearrange(
                        "p (a b) -> p a b", a=8, b=128)
                    for k in range(8):
                        dp = db * 8 + k
                        nc.tensor.matmul(out=ps[:, k, :],
                                         lhsT=t1[v][:, :, dp],
                                         rhs=bw[:, :])
                    dst = t2[v][:, db * 8:db * 8 + 8, :]
                    if evac_on_act(nev):
                        nc.scalar.copy(dst, ps[:, :, :])
                    else:
                        nc.vector.tensor_copy(dst, ps[:, :, :])
                    nev += 1

            # Early per-chunk stage: psI/psJ + ap/bp + qI/qJ/m only needs
            # the I and J volumes' t2 -> hoisted into the P_D/P_W stream.
            qtiles = {}

            def early_chunk(c):
                rhsI = t2[0][:, c * 4:c * 4 + 4, :].rearrange(
                    "p a b -> p (a b)")
                rhsJ = t2[1][:, c * 4:c * 4 + 4, :].rearrange(
                    "p a b -> p (a b)")
                ps = psDp.tile([128, 1024], F32, tag="psD", name="psij")
                nc.tensor.matmul(out=ps[:, 0:512], lhsT=bw[:, :], rhs=rhsI)
                nc.tensor.matmul(out=ps[:, 512:1024], lhsT=bw[:, :], rhs=rhsJ)
                ap = ptw2.tile([128, 512], BF16, tag="ap", name="ap")
                bp = ptw2.tile([128, 512], BF16, tag="bp", name="bp")
                nc.scalar.mul(ap[:, :], ps[:, 0:512], SQS)
                nc.scalar.mul(bp[:, :], ps[:, 512:1024], SQS)
                qI = espp.tile([128, 512], BF16, tag="qI", name="qI")
                qJ = espp.tile([128, 512], BF16, tag="qJ", name="qJ")
                m = espp.tile([128, 512], BF16, tag="m", name="m")
                nc.gpsimd.tensor_tensor(out=qI[:, :], in0=ap[:, :],
                                        in1=ap[:, :], op=ALU.mult)
                nc.vector.tensor_tensor(out=qJ[:, :], in0=bp[:, :],
                                        in1=bp[:, :], op=ALU.mult)
                nc.gpsimd.tensor_tensor(out=m[:, :], in0=ap[:, :],
                                        in1=bp[:, :], op=ALU.mult)
                qtiles[c] = (qI, qJ, m)

            # Raw inputs first (only DMA-gated), products trail their prep.
            p_d(0, vi)
            p_d(1, vj)
            p_w(0)
            for q in range(4):
                s = slice(q * 16, q * 16 + 16)
                nc.scalar.square(vi2[:, s, :], vi[:, s, :])
                nc.vector.tensor_tensor(out=vj2[:, s, :], in0=vj[:, s, :],
                                        in1=vj[:, s, :], op=ALU.mult)
                nc.gpsimd.tensor_tensor(out=vij[:, s, :], in0=vi[:, s, :],
                                        in1=vj[:, s, :], op=ALU.mult)
            p_d(2, vi2)
            p_w(1)
            early_chunk(0)
            early_chunk(1)
            p_d(3, vj2)
            p_w(2)
            early_chunk(2)
            early_chunk(3)
            p_d(4, vij)
            p_w(3)
            early_chunk(4)
            early_chunk(5)
            p_w(4)
            early_chunk(6)
            early_chunk(7)

            volp.__exit__(None, None, None)

            # ---------- P_W + P_H + pointwise, pipelined per d'-block ----
            # P_W ordered d'-block-outer so chunk c's P_H + pointwise can
            # trail one block behind P_W(c+1): Pool's pointwise overlaps
            # ACT/DVE evacuations, P_W matmuls fill PE between P_H chunks.
            ptwp = tc.tile_pool(name="ptw", bufs=4)
            ptw = ptwp.__enter__()

            accs = accp.tile([128, 8], F32)
            pend = {}

            def emit_chunk(c):
                rhs = [t2[v][:, c * 4:c * 4 + 4, :].rearrange(
                    "p a b -> p (a b)") for v in (2, 3, 4)]
                qI, qJ, m = qtiles[c]
                tA = psDp.tile([128, 1024], F32, tag="psD", name="psA")
                tB = psDp.tile([128, 1024], F32, tag="psD", name="psB")
                psI2 = tA[:, 0:512]
                psJ2 = tA[:, 512:1024]
                psIJ = tB[:, 0:512]
                nc.tensor.matmul(out=psI2[:, :], lhsT=bw[:, :], rhs=rhs[0],
                                 start=True, stop=False)
                nc.tensor.matmul(out=psI2[:, :], lhsT=negI[:, :], rhs=qI[:, :],
                                 start=False, stop=True)
                nc.tensor.matmul(out=psJ2[:, :], lhsT=bw[:, :], rhs=rhs[1],
                                 start=True, stop=False)
                nc.tensor.matmul(out=psJ2[:, :], lhsT=negI[:, :], rhs=qJ[:, :],
                                 start=False, stop=True)
                nc.tensor.matmul(out=psIJ[:, :], lhsT=bw[:, :], rhs=rhs[2],
                                 start=True, stop=False)
                nc.tensor.matmul(out=psIJ[:, :], lhsT=negI[:, :], rhs=m[:, :],
                                 start=False, stop=True)
                pend[c] = (psI2, psJ2, psIJ)

            def finish_chunk(c):
                psI2, psJ2, psIJ = pend.pop(c)
                qtiles.pop(c)
                ivp = ptw.tile([128, 512], BF16, tag="ivp", name="ivp")
                crp = ptw.tile([128, 512], BF16, tag="crp", name="crp")
                nc.scalar.copy(ivp[:, :], psI2[:, :])
                nc.scalar.copy(crp[:, :], psIJ[:, :])

                den = ptw.tile([128, 512], F32, tag="den", name="den")
                nc.vector.tensor_tensor(out=den[:, :], in0=psJ2[:, :],
                                        in1=ivp[:, :], op=ALU.mult)
                rec = ptw.tile([128, 512], F32, tag="rec", name="rec")
                nc.vector.reciprocal_approx_fast(out=rec[:, :], in_=den[:, :])

                cc = ptw.tile([128, 512], F32, tag="cc", name="cc")
                nc.vector._custom_dve(CC_SQ_ANT, out=cc[:, :], in0=rec[:, :],
                                      in1=crp[:, :],
                                      accum_out=accs[:, c:c + 1])

            for c in range(9):
                if c < 8:
                    emit_chunk(c)
                if c >= 1:
                    finish_chunk(c - 1)

            nc.sync.dma_start(out=out_dram[:, :], in_=accs[:, :])
            ptwp.__exit__(None, None, None)
            psD.__exit__(None, None, None)
            ptw2p.__exit__(None, None, None)
            esp.__exit__(None, None, None)
            t1p.__exit__(None, None, None)

    nc.compile()
    return nc


def kernel(y_pred: np.ndarray, y_true: np.ndarray) -> np.ndarray:
    y_pred = np.asarray(y_pred, dtype=np.float32)
    y_true = np.asarray(y_true, dtype=np.float32)

    if "nc" not in _CACHE:
        _CACHE["nc"] = _build()
    nc = _CACHE["nc"]

    ib = y_true.astype(ml_dtypes.bfloat16)
    jb = y_pred.astype(ml_dtypes.bfloat16)

    in_maps = []
    for core in range(8):
        b = core // 4
        d0 = (core % 4) * DL
        lo, hi = d0 - PAD, d0 + DL + PAD
        slo, shi = max(lo, 0), min(hi, D)
        ipk = np.zeros((104, 64, 128), ml_dtypes.bfloat16)
        jpk = np.zeros((104, 64, 128), ml_dtypes.bfloat16)
        for hb in range(2):
            hs = slice(hb * 64, hb * 64 + 64)
            p0 = 64 * hb
            ipk[p0 + slo - lo:p0 + shi - lo] = ib[b, 0, slo:shi, hs, :]
            jpk[p0 + slo - lo:p0 + shi - lo] = jb[b, 0, slo:shi, hs, :]
        in_maps.append({"i_pk": ipk, "j_pk": jpk})

    res = run_bass_kernel_spmd(nc, in_maps, core_ids=list(range(8)))
    total = 0.0
    for r in res.results:
        total += float(np.asarray(r["partials"], np.float64).sum())
    return np.float32(-total / N_TOT)


if __name__ == "__main__":
    rng = np.random.default_rng(0)
    yp = rng.standard_normal((B, 1, D, H, W), dtype=np.float32)
    yt = rng.standard_normal((B, 1, D, H, W), dtype=np.float32)
    print("loss:", kernel(yp, yt))
